# revision 10
# baseline (speedup 1.0000x reference)
"""GQA multi-head attention (B=1, S=4096, E=2048, H=16, HK=4, D=128) on 8 trn2
NeuronCores.

Sharding: tensor-parallel over query heads — 2 q-heads per core, each core
also computes the kv head its q-heads attend to (each kv head is replicated
on the 2 cores that need it). Each core produces a partial output
y_c = attn_c @ Wo_c and the host sums the 8 partials during unsharding
(so the device program needs no collectives).

Device-side dataflow per core (matmul inputs fp16, accumulation fp32):
  xT [E,S] -> qT [D,h,S], kT [D,S] (transposed projections), v [S,D]
  scoresT[t,sq] = (kT chunk as lhsT).T @ qT      (t-chunk on partitions)
  pT = exp(scoresT/sqrt(D)) via ACT -> fp16
  outT[d,sq] accumulated over t-chunks: lhsT=v[t,d], rhs=pT[t,sq]
  rowsums: DVE adds over t-chunks, then ones-matmul partition-sum+broadcast
  attnT = outT * (1/rowsum); o_proj: y[s,e] = (attnT as lhsT).T @ WoT
"""
import math
import numpy as np
from contextlib import ExitStack

import concourse.bass as bass
import concourse.mybir as mybir
from concourse import tile
from concourse import bass_utils

B, S, E = 1, 4096, 2048
H, HK, D = 16, 4, 128
N_CORES = 8
HPC = H // N_CORES          # q heads per core
QDIM = HPC * D              # 256
EC = E // 128               # e-chunks
SB = 512                    # s/sq block
NSB = S // SB
TC = S // 128               # t-chunks
SCALE = 1.0 / math.sqrt(D)
FP16 = mybir.dt.float16
FP32 = mybir.dt.float32


def _split_sync_waits(nc, cap=1):
    """This container's walrus build rejects instructions carrying more than
    ~1 sync-wait (codegen 'Too many sync wait commands'). Post-pass over the
    scheduled BIR: for any instruction with >cap waits, hoist the excess onto
    same-engine NOPs inserted immediately before it (same block, so per-engine
    program order — and therefore semantics — is preserved)."""
    n = 0
    for fn in nc.m.functions:
        for blk in fn.blocks:
            il = blk.instructions
            i = 0
            while i < len(il):
                inst = il[i]
                si = getattr(inst, "sync_info", None)
                if si is not None and len(si.on_wait) > cap:
                    waits = list(si.on_wait)
                    si.on_wait = waits[-cap:]
                    extras = []
                    for w in waits[:-cap]:
                        nop = mybir.InstNoOp(name=f"I-waitfix-{n}", ins=[], outs=[])
                        n += 1
                        nop.engine = inst.engine
                        nop.sync_info = mybir.SyncInfo(on_wait=[w], on_update=[])
                        extras.append(nop)
                    il[i:i] = extras
                    i += len(extras)
                i += 1
    return n


XTW = 2048                  # xt tile width (half of S per tile)
NHALF = S // XTW            # 2 halves


def _emit_program(nc, tc, aps, weights, r):
    """Emit one full forward pass. `r` suffixes pool/tile names so the
    program can be repeated for timing calibration.

    The PE executes its instruction stream in order, so emission order is
    the PE schedule. The attention inner loop is software-pipelined
    (QK of chunk tp+1 is emitted before PV of chunk tp, so the PE never
    waits on the exp of the chunk it just produced), and independent
    matmuls (q-projection of the next query block, output projection of
    the previous one, rowsum matmuls) are injected as fillers into the
    slack the ACT-paced exp stream leaves on the PE.

    PSUM budget (8 banks): kv/y shared tag 1, q 1, sps 2x2, osum 2x1."""
    xT, y = aps
    wq_sb, wk_sb, wv_sb, wo_sb, ones_sb = weights

    big = tc.alloc_tile_pool(name=f"big{r}", bufs=1)
    qT_sb = big.tile([128, HPC, S], FP16, name=f"qT{r}")   # [d, h, s]
    kT_sb = big.tile([128, S], FP16, name=f"kT{r}")        # [d, t]
    v_sb = big.tile([128, S], FP16, name=f"v{r}")          # [t%128, tc*128+d]
    aT_sb = big.tile([128, HPC, S], FP16, name=f"aT{r}")   # [d, h, s]

    with ExitStack() as ctx:
        xpool = ctx.enter_context(tc.tile_pool(name=f"xpool{r}", bufs=24))
        ps = ctx.enter_context(tc.tile_pool(name=f"ps{r}", bufs=1, space="PSUM"))
        ptp = ctx.enter_context(tc.tile_pool(name=f"ptp{r}", bufs=4))
        accp = ctx.enter_context(tc.tile_pool(name=f"accp{r}", bufs=2))
        y_sbp = ctx.enter_context(tc.tile_pool(name=f"y_sbp{r}", bufs=2))

        xt_tiles = {}
        HW = XTW // 2           # column half of one xt tile (2 sb blocks)

        def load_half_cols(phase, half, part, eng):
            """DMA the `part` column-half of all EC tiles of (phase, half),
            in ec order — i.e. in PE consumption order, so the k/v/q matmul
            ec-sweep never waits on data that arrives later than needed.
            `eng` picks the issuing engine: DMA issue costs ~0.8us per slice
            on the issuing engine, so x streaming is spread across SP and
            ACT (idle during phase A) to keep issue off the critical path."""
            for ec in range(EC):
                key = (phase, half, ec)
                if key not in xt_tiles:
                    xt_tiles[key] = xpool.tile(
                        [128, XTW], FP16,
                        name=f"xt{r}{phase}_{half}_{ec}", tag="xt")
                src0 = half * XTW + part * HW
                eng.dma_start(
                    xt_tiles[key][:, part * HW:(part + 1) * HW],
                    xT[ec * 128:(ec + 1) * 128, src0:src0 + HW])

        def xt_slice(phase, sb, ec, width=SB, sub=0):
            half, off = divmod(sb * SB + sub, XTW)
            return xt_tiles[(phase, half, ec)][:, off:off + width]

        # ---- Phase A: k/v projections ----
        # Consumption-ordered DMAs: first wk chunk first (the first matmul
        # needs only wk[:, 0:4, :]), x column-halves in sbp consumption
        # order, dual-issued on SP + ACT.
        if r == 0:
            wdmas = _WEIGHT_DMAS.pop(0)
            (wk_dst, wk_src), (wv_dst, wv_src) = wdmas["kv"]
            (wq_dst, wq_src), (wo_dst, wo_src) = wdmas["qo"]
            for g in range(0, EC, 4):       # wk in 4-ec chunks
                nc.sync.dma_start(wk_dst[:, g:g + 4, :], wk_src[:, g:g + 4, :])
            for g in range(0, EC, 8):       # wv in 8-ec chunks
                nc.sync.dma_start(wv_dst[:, g:g + 8, :], wv_src[:, g:g + 8, :])
        load_half_cols("a", 0, 0, nc.scalar)    # sbp 0
        load_half_cols("a", 0, 1, nc.sync)      # sbp 1

        prev_v = None           # (v_ps, sb0) whose jj-groups are pending

        def v_group(vps, vsb0, jj):
            """Full ec-accumulation of one 128-col v output block; evict
            after the last block."""
            sb = vsb0 + jj // (SB // 128)
            sub = (jj % (SB // 128)) * 128
            for ec in range(EC):
                nc.tensor.matmul(
                    vps[:, jj * 128:(jj + 1) * 128],
                    xt_slice("a", sb, ec, width=128, sub=sub),
                    wv_sb[:, ec, :],
                    start=(ec == 0), stop=(ec == EC - 1))
            if jj == 2 * SB // 128 - 1:
                nc.scalar.copy(v_sb[:, vsb0 * SB:(vsb0 + 2) * SB], vps[:])

        for sbp in range(NSB // 2):
            if sbp == 1:
                load_half_cols("a", 1, 0, nc.scalar)
                load_half_cols("a", 1, 1, nc.sync)
            if sbp == 2:
                if r == 0:
                    for g in range(0, EC, 8):
                        nc.scalar.dma_start(wq_dst[:, g:g + 8, :],
                                            wq_src[:, g:g + 8, :])
                load_half_cols("b", 0, 0, nc.sync)
                load_half_cols("b", 0, 1, nc.sync)
            if sbp == 3:
                if r == 0:
                    nc.sync.dma_start(wo_dst, wo_src)
                load_half_cols("b", 1, 0, nc.sync)
                load_half_cols("b", 1, 1, nc.sync)
            sb0, sb1 = 2 * sbp, 2 * sbp + 1
            # v(sbp-1) interleaves with k(sbp): k is the consumer of fresh
            # x slices (0.43us PE per 0.25MB -> ~580GB/s if run alone); the
            # trailing v re-reads the previous sbp's tiles, smoothing fresh-
            # data demand to ~290GB/s. v's jj accumulation groups stay
            # SEQUENTIAL — concurrent groups within one 512-col psum zero
            # region are illegal (PE pending-group rule) and corrupt data.
            k_ps = ps.tile([128, 2 * SB], FP32, name=f"kps{r}_{sbp}",
                           tag="sps", bufs=2)
            v_ps = ps.tile([128, 2 * SB], FP32, name=f"vps{r}_{sbp}",
                           tag="sps", bufs=2)
            for ec in range(EC):
                nc.tensor.matmul(k_ps[:, 0:SB], wk_sb[:, ec, :],
                                 xt_slice("a", sb0, ec),
                                 start=(ec == 0), stop=(ec == EC - 1))
                nc.tensor.matmul(k_ps[:, SB:2 * SB], wk_sb[:, ec, :],
                                 xt_slice("a", sb1, ec),
                                 start=(ec == 0), stop=(ec == EC - 1))
                if prev_v is not None and ec % 2 == 1:
                    v_group(*prev_v, ec // 2)
            nc.vector.tensor_copy(
                kT_sb[:, sb0 * SB:(sb0 + 2) * SB], k_ps[:])
            prev_v = (v_ps, sb0)
        for jj in range(2 * SB // 128):     # trailing v for the last sbp
            v_group(*prev_v, jj)



        def q_proj_closures(qb):
            """One closure per matmul of q(qb); evicts attached to the last."""
            cls = []
            for ic in range(HPC):
                q_ps = ps.tile([128, SB], FP32, name=f"qps{r}_{qb}_{ic}",
                               tag="q")

                def mk(ic, ec, q_ps):
                    def emit():
                        nc.tensor.matmul(
                            q_ps[:],
                            wq_sb[:, ec, ic * 128:(ic + 1) * 128],
                            xt_slice("b", qb, ec),
                            start=(ec == 0), stop=(ec == EC - 1))
                        if ec == EC - 1:
                            nc.vector.tensor_copy(
                                qT_sb[:, ic, qb * SB:(qb + 1) * SB], q_ps[:])
                    return emit
                for ec in range(EC):
                    cls.append(mk(ic, ec, q_ps))
            return cls

        n_y = [0]

        def o_proj_closures(qb, tag="kv"):
            """One closure per (sc, eb): 2 matmuls + evict; store per sc."""
            cls = []
            for sc in range(qb * (SB // 128), (qb + 1) * (SB // 128)):
                y_t = y_sbp.tile([128, E], FP16, name=f"ysb{r}_{sc}",
                                 tag="ysb")
                for eb in range(E // SB):
                    def mk(sc, eb, y_t):
                        def emit():
                            shape = [128, 2 * SB] if tag == "sps" else [128, SB]
                            y_ps = ps.tile(shape, FP32,
                                           name=f"yps{r}_{sc}_{eb}", tag=tag,
                                           bufs=2 if tag == "sps" else None)
                            for h in range(HPC):
                                nc.tensor.matmul(
                                    y_ps[:, 0:SB],
                                    aT_sb[:, h, sc * 128:(sc + 1) * 128],
                                    wo_sb[:, h, eb * SB:(eb + 1) * SB],
                                    start=(h == 0), stop=(h == HPC - 1))
                            nc.vector.tensor_copy(
                                y_t[:, eb * SB:(eb + 1) * SB], y_ps[:, 0:SB])
                            n_y[0] += 1
                            if sc == S // 128 - 1:
                                # last row block: store per-eb so the final
                                # DMA is small and the drain barrier clears
                                # sooner
                                nc.sync.dma_start(
                                    y[sc * 128:(sc + 1) * 128,
                                      eb * SB:(eb + 1) * SB],
                                    y_t[:, eb * SB:(eb + 1) * SB])
                            elif eb == E // SB - 1:
                                nc.sync.dma_start(
                                    y[sc * 128:(sc + 1) * 128, :], y_t[:])
                        return emit
                    cls.append(mk(sc, eb, y_t))
            return cls

        def finish_head_closure(h, qb, o_ps, sums):
            def emit():
                sums_ps = ps.tile([128, SB], FP32,
                                  name=f"sums_ps{r}_{h}_{qb}", tag="q")
                nc.tensor.matmul(sums_ps[:], ones_sb[:], sums[:],
                                 start=True, stop=True)
                recip = accp.tile([128, SB], FP32, name=f"recip{r}_{h}_{qb}",
                                  tag="recip")
                nc.vector.reciprocal(recip[:], sums_ps[:])
                nc.vector.tensor_mul(
                    aT_sb[:, h, qb * SB:(qb + 1) * SB], o_ps[:], recip[:])
            return emit

        # ---- Phase B: attention, with q/o-projection work as PE fillers ----
        from collections import deque
        fillers = deque()
        q0 = q_proj_closures(0)
        for c in q0[:EC]:        # q(0, head 0) must precede attention(h=0)
            c()
        fillers.extend(q0[EC:])  # q(0, head 1) fills attention(h=0) slack
        pending_oproj = []       # o_proj of qb becomes available after qb

        for qb in range(NSB):
            if qb + 1 < NSB:
                fillers.extend(q_proj_closures(qb + 1))
            fillers.extend(pending_oproj)
            pending_oproj = []
            for h in range(HPC):
                o_ps = ps.tile([128, SB], FP32, name=f"ops{r}_{h}_{qb}",
                               tag="osum", bufs=2)
                acc2 = accp.tile([128, 2 * SB], FP16, name=f"acc{r}_{h}_{qb}",
                                 tag="acc")

                def qk(tp):
                    s_ps = ps.tile([128, 2 * SB], FP32,
                                   name=f"sps{r}_{h}_{qb}_{tp}", tag="sps",
                                   bufs=2)
                    for hf in range(2):
                        t = tp * 2 + hf
                        nc.tensor.matmul(
                            s_ps[:, hf * SB:(hf + 1) * SB],
                            kT_sb[:, t * 128:(t + 1) * 128],
                            qT_sb[:, h, qb * SB:(qb + 1) * SB],
                            start=True, stop=True)
                    return s_ps

                s_prev = qk(0)
                pt_first = None
                for tp in range(TC // 2):
                    pt = ptp.tile([128, 2 * SB], FP16,
                                  name=f"pt{r}_{h}_{qb}_{tp}", tag="pt")
                    nc.scalar.activation(
                        pt[:], s_prev[:],
                        mybir.ActivationFunctionType.Exp, scale=SCALE)
                    if tp + 1 < TC // 2:
                        s_prev = qk(tp + 1)
                    for hf in range(2):
                        t = tp * 2 + hf
                        nc.tensor.matmul(
                            o_ps[:],
                            v_sb[:, t * 128:(t + 1) * 128],
                            pt[:, hf * SB:(hf + 1) * SB],
                            start=(t == 0), stop=(t == TC - 1))
                    if tp == 0:
                        pt_first = pt
                    elif tp == 1:
                        nc.vector.tensor_add(acc2[:], pt_first[:], pt[:])
                    else:
                        nc.vector.tensor_add(acc2[:], acc2[:], pt[:])
                    slots_left = (HPC - h) * (TC // 2) - tp
                    floor = 2 if (qb == 0 and h == 0) else 1
                    n_pop = min(len(fillers),
                                max(floor,
                                    -(-len(fillers) // max(slots_left, 1))),
                                3)
                    for _ in range(n_pop):
                        if fillers:
                            fillers.popleft()()
                sums = accp.tile([128, SB], FP16, name=f"sums{r}_{h}_{qb}",
                                 tag="sums")
                nc.vector.tensor_add(sums[:], acc2[:, 0:SB], acc2[:, SB:2 * SB])
                fillers.append(finish_head_closure(h, qb, o_ps, sums))
            pending_oproj = o_proj_closures(
                qb, tag="sps" if qb == NSB - 1 else "kv")

        while fillers:
            fillers.popleft()()
        for c in pending_oproj:
            c()

    big.release()


_WEIGHT_DMAS = []


def build_bass(reps=1):
    nc = bass.Bass("TRN2", target_bir_lowering=False, debug=False,
                   num_devices=N_CORES)
    xT = nc.dram_tensor("xT", [E, S], FP16, kind="ExternalInput").ap()
    wq = nc.dram_tensor("wq", [E, QDIM], FP16, kind="ExternalInput").ap()
    wk = nc.dram_tensor("wk", [E, D], FP16, kind="ExternalInput").ap()
    wv = nc.dram_tensor("wv", [E, D], FP16, kind="ExternalInput").ap()
    wo = nc.dram_tensor("wo", [QDIM, E], FP16, kind="ExternalInput").ap()
    y = nc.dram_tensor("y", [S, E], FP16, kind="ExternalOutput").ap()

    with tile.TileContext(nc) as tc, ExitStack() as ctx:
        wpool = ctx.enter_context(tc.tile_pool(name="wpool", bufs=1))
        wq_sb = wpool.tile([128, EC, QDIM], FP16)
        wk_sb = wpool.tile([128, EC, D], FP16)
        wv_sb = wpool.tile([128, EC, D], FP16)
        wo_sb = wpool.tile([128, HPC, E], FP16)
        ones_sb = wpool.tile([128, 128], FP16)
        nc.vector.memset(ones_sb[:], 1.0)
        kv_dmas = [
            (wk_sb[:], wk.rearrange("(ec p) d -> p ec d", p=128)),
            (wv_sb[:], wv.rearrange("(ec p) d -> p ec d", p=128)),
        ]
        qo_dmas = [
            (wq_sb[:], wq.rearrange("(ec p) d -> p ec d", p=128)),
            (wo_sb[:], wo.rearrange("(h p) e -> p h e", p=128)),
        ]
        _WEIGHT_DMAS.clear()
        _WEIGHT_DMAS.append({"kv": kv_dmas, "qo": qo_dmas})

        for r in range(reps):
            _emit_program(nc, tc, (xT, y), (wq_sb, wk_sb, wv_sb, wo_sb, ones_sb), r)

    _split_sync_waits(nc)
    return nc


def make_in_maps(x, Wq, Wk, Wv, Wo):
    """Host-side sharding: transpose/cast to fp16, slice weights per core."""
    x = np.asarray(x, dtype=np.float32).reshape(S, E)
    xT = np.ascontiguousarray(x.T).astype(np.float16)
    WqT = np.ascontiguousarray(np.asarray(Wq, dtype=np.float32).T).astype(np.float16)
    WkT = np.ascontiguousarray(np.asarray(Wk, dtype=np.float32).T).astype(np.float16)
    WvT = np.ascontiguousarray(np.asarray(Wv, dtype=np.float32).T).astype(np.float16)
    WoT = np.ascontiguousarray(np.asarray(Wo, dtype=np.float32).T).astype(np.float16)
    in_maps = []
    for c in range(N_CORES):
        g = (c * HPC) // (H // HK)      # kv head for this core's q heads
        in_maps.append({
            "xT": xT,
            "wq": np.ascontiguousarray(WqT[:, c * QDIM:(c + 1) * QDIM]),
            "wk": np.ascontiguousarray(WkT[:, g * D:(g + 1) * D]),
            "wv": np.ascontiguousarray(WvT[:, g * D:(g + 1) * D]),
            "wo": np.ascontiguousarray(WoT[c * QDIM:(c + 1) * QDIM, :]),
        })
    return in_maps


class Runner:
    """Compile the bass program once and keep the jitted SPMD executable
    alive, so repeated kernel()/timing calls skip re-trace + re-compile
    (the stock run_bass_kernel_spmd path builds a fresh closure per call,
    costing ~10 s each). Mirrors bass2jax.run_bass_via_pjrt's staging."""

    def __init__(self, reps=1):
        import jax
        from jax.sharding import PartitionSpec, Mesh, NamedSharding
        from jax.experimental.shard_map import shard_map
        from concourse import bass2jax

        self.jax = jax
        nc = build_bass(reps)
        self.nc = nc
        bass2jax.install_neuronx_cc_hook()
        partition_name = (nc.partition_id_tensor.name
                          if nc.partition_id_tensor else None)
        in_names, out_names, out_avals, zero_outs = [], [], [], []
        for alloc in nc.m.functions[0].allocations:
            if not isinstance(alloc, mybir.MemoryLocationSet):
                continue
            name = alloc.memorylocations[0].name
            if alloc.kind == "ExternalInput":
                if name != partition_name:
                    in_names.append(name)
            elif alloc.kind == "ExternalOutput":
                out_names.append(name)
                shape = tuple(alloc.tensor_shape)
                dtype = mybir.dt.np(alloc.dtype)
                out_avals.append(jax.core.ShapedArray(shape, dtype))
                zero_outs.append(np.zeros(shape, dtype))
        self.in_names, self.out_names = in_names, out_names
        self.zero_outs = zero_outs
        all_names = in_names + out_names
        if partition_name is not None:
            all_names = all_names + [partition_name]

        def _body(*args):
            operands = list(args)
            if partition_name is not None:
                operands.append(bass2jax.partition_id_tensor())
            outs = bass2jax._bass_exec_p.bind(
                *operands,
                out_avals=tuple(out_avals),
                in_names=tuple(all_names),
                out_names=tuple(out_names),
                lowering_input_output_aliases=(),
                sim_require_finite=True,
                sim_require_nnan=True,
                nc=nc,
            )
            return tuple(outs)

        devices = jax.devices()[:N_CORES]
        mesh = Mesh(np.asarray(devices), ("core",))
        spec = PartitionSpec("core")
        self.sharding = NamedSharding(mesh, spec)
        self.fn = jax.jit(shard_map(_body, mesh=mesh, in_specs=spec,
                                    out_specs=spec, check_rep=False))

    def stage(self, in_maps):
        """Concat per-core inputs on axis 0 and put on the 8 devices."""
        global_ins = [
            np.concatenate([np.asarray(m[name]) for m in in_maps], axis=0)
            for name in self.in_names
        ] + [
            np.concatenate([z] * N_CORES, axis=0) for z in self.zero_outs
        ]
        return [self.jax.device_put(a, self.sharding) for a in global_ins]

    def run_staged(self, dev_ins):
        return self.fn(*dev_ins)

    def run(self, in_maps):
        outs = self.fn(*self.stage(in_maps))
        return [
            {name: np.asarray(outs[i]).reshape(N_CORES, *self.zero_outs[i].shape)[c]
             for i, name in enumerate(self.out_names)}
            for c in range(N_CORES)
        ]


_RUNNER_CACHE = {}


def get_runner(reps=1):
    if reps not in _RUNNER_CACHE:
        _RUNNER_CACHE[reps] = Runner(reps)
    return _RUNNER_CACHE[reps]


def get_nc():
    return get_runner(1).nc


def kernel(x, Wq, Wk, Wv, Wo):
    runner = get_runner(1)
    in_maps = make_in_maps(x, Wq, Wk, Wv, Wo)
    res = runner.run(in_maps)
    out = np.zeros((S, E), dtype=np.float32)
    for r in res:
        out += r["y"].astype(np.float32)
    return out.reshape(B, S, E)



# revision 19
# speedup vs baseline: 1.0046x; 1.0046x over previous
"""GQA multi-head attention (B=1, S=4096, E=2048, H=16, HK=4, D=128) on 8 trn2
NeuronCores.

Sharding: tensor-parallel over query heads — 2 q-heads per core, each core
also computes the kv head its q-heads attend to (each kv head is replicated
on the 2 cores that need it). Each core produces a partial output
y_c = attn_c @ Wo_c and the host sums the 8 partials during unsharding
(so the device program needs no collectives).

Device-side dataflow per core (matmul inputs fp16, accumulation fp32):
  xT [E,S] -> qT [D,h,S], kT [D,S] (transposed projections), v [S,D]
  scoresT[t,sq] = (kT chunk as lhsT).T @ qT      (t-chunk on partitions)
  pT = exp(scoresT/sqrt(D)) via ACT -> fp16
  outT[d,sq] accumulated over t-chunks: lhsT=v[t,d], rhs=pT[t,sq]
  rowsums: DVE adds over t-chunks, then ones-matmul partition-sum+broadcast
  attnT = outT * (1/rowsum); o_proj: y[s,e] = (attnT as lhsT).T @ WoT
"""
import math
import numpy as np
from contextlib import ExitStack

import concourse.bass as bass
import concourse.mybir as mybir
from concourse import tile
from concourse import bass_utils
from concourse import masks

B, S, E = 1, 4096, 2048
H, HK, D = 16, 4, 128
N_CORES = 8
HPC = H // N_CORES          # q heads per core
QDIM = HPC * D              # 256
EC = E // 128               # e-chunks
SB = 512                    # s/sq block
NSB = S // SB
TC = S // 128               # t-chunks
SCALE = 1.0 / math.sqrt(D)
FP16 = mybir.dt.float16
FP32 = mybir.dt.float32


def _split_sync_waits(nc, cap=1):
    """This container's walrus build rejects instructions carrying more than
    ~1 sync-wait (codegen 'Too many sync wait commands'). Post-pass over the
    scheduled BIR: for any instruction with >cap waits, hoist the excess onto
    same-engine NOPs inserted immediately before it (same block, so per-engine
    program order — and therefore semantics — is preserved)."""
    n = 0
    for fn in nc.m.functions:
        for blk in fn.blocks:
            il = blk.instructions
            i = 0
            while i < len(il):
                inst = il[i]
                si = getattr(inst, "sync_info", None)
                if si is not None and len(si.on_wait) > cap:
                    waits = list(si.on_wait)
                    si.on_wait = waits[-cap:]
                    extras = []
                    for w in waits[:-cap]:
                        nop = mybir.InstNoOp(name=f"I-waitfix-{n}", ins=[], outs=[])
                        n += 1
                        nop.engine = inst.engine
                        nop.sync_info = mybir.SyncInfo(on_wait=[w], on_update=[])
                        extras.append(nop)
                    il[i:i] = extras
                    i += len(extras)
                i += 1
    return n


XTW = 2048                  # xt tile width (half of S per tile)
NHALF = S // XTW            # 2 halves


def _emit_program(nc, tc, aps, weights, r):
    """Emit one full forward pass. `r` suffixes pool/tile names so the
    program can be repeated for timing calibration.

    The PE executes its instruction stream in order, so emission order is
    the PE schedule. The attention inner loop is software-pipelined
    (QK of chunk tp+1 is emitted before PV of chunk tp, so the PE never
    waits on the exp of the chunk it just produced), and independent
    matmuls (q-projection of the next query block, output projection of
    the previous one, rowsum matmuls) are injected as fillers into the
    slack the ACT-paced exp stream leaves on the PE.

    PSUM budget (8 banks): kv/y shared tag 1, q 1, sps 2x2, osum 2x1."""
    xT, y = aps
    wq_sb, wk_sb, wv_sb, wo_sb, ones_sb, ident_sb = weights

    big = tc.alloc_tile_pool(name=f"big{r}", bufs=1)
    qT_sb = big.tile([128, HPC, S], FP16, name=f"qT{r}")   # [d, h, s]
    kT_sb = big.tile([128, S], FP16, name=f"kT{r}")        # [d, t]
    v_sb = big.tile([128, S], FP16, name=f"v{r}")          # [t%128, tc*128+d]
    aT_sb = big.tile([128, HPC, S], FP16, name=f"aT{r}")   # [d, h, s]

    with ExitStack() as ctx:
        xpool = ctx.enter_context(tc.tile_pool(name=f"xpool{r}", bufs=24))
        ps = ctx.enter_context(tc.tile_pool(name=f"ps{r}", bufs=1, space="PSUM"))
        ptp = ctx.enter_context(tc.tile_pool(name=f"ptp{r}", bufs=4))
        accp = ctx.enter_context(tc.tile_pool(name=f"accp{r}", bufs=2))
        y_sbp = ctx.enter_context(tc.tile_pool(name=f"y_sbp{r}", bufs=2))
        vtp = ctx.enter_context(tc.tile_pool(name=f"vtp{r}", bufs=2))

        xt_tiles = {}
        HW = XTW // 2           # column half of one xt tile (2 sb blocks)

        def load_half_cols(phase, half, part, eng):
            """DMA the `part` column-half of all EC tiles of (phase, half),
            in ec order — i.e. in PE consumption order, so the k/v/q matmul
            ec-sweep never waits on data that arrives later than needed.
            `eng` picks the issuing engine: DMA issue costs ~0.8us per slice
            on the issuing engine, so x streaming is spread across SP and
            ACT (idle during phase A) to keep issue off the critical path."""
            for ec in range(EC):
                key = (phase, half, ec)
                if key not in xt_tiles:
                    xt_tiles[key] = xpool.tile(
                        [128, XTW], FP16,
                        name=f"xt{r}{phase}_{half}_{ec}", tag="xt")
                src0 = half * XTW + part * HW
                eng.dma_start(
                    xt_tiles[key][:, part * HW:(part + 1) * HW],
                    xT[ec * 128:(ec + 1) * 128, src0:src0 + HW])

        def xt_slice(phase, sb, ec, width=SB, sub=0):
            half, off = divmod(sb * SB + sub, XTW)
            return xt_tiles[(phase, half, ec)][:, off:off + width]

        # ---- Phase A: k/v projections ----
        # Consumption-ordered DMAs: first wk chunk first (the first matmul
        # needs only wk[:, 0:4, :]), x column-halves in sbp consumption
        # order, dual-issued on SP + ACT.
        if r == 0:
            wdmas = _WEIGHT_DMAS.pop(0)
            (wk_dst, wk_src), (wv_dst, wv_src) = wdmas["kv"]
            (wq_dst, wq_src), (wo_dst, wo_src) = wdmas["qo"]
            for g in range(0, EC, 4):       # wk in 4-ec chunks
                nc.sync.dma_start(wk_dst[:, g:g + 4, :], wk_src[:, g:g + 4, :])
            for g in range(0, EC, 8):       # wv in 8-ec chunks
                nc.sync.dma_start(wv_dst[:, g:g + 8, :], wv_src[:, g:g + 8, :])
        load_half_cols("a", 0, 0, nc.scalar)    # sbp 0
        load_half_cols("a", 0, 1, nc.sync)      # sbp 1

        for sbp in range(NSB // 2):
            if sbp == 1:
                load_half_cols("a", 1, 0, nc.scalar)
                load_half_cols("a", 1, 1, nc.sync)
            if sbp == 2:
                if r == 0:
                    for g in range(0, EC, 8):
                        nc.scalar.dma_start(wq_dst[:, g:g + 8, :],
                                            wq_src[:, g:g + 8, :])
                load_half_cols("b", 0, 0, nc.sync)
                load_half_cols("b", 0, 1, nc.sync)
            if sbp == 3:
                if r == 0:
                    nc.sync.dma_start(wo_dst, wo_src)
                load_half_cols("b", 1, 0, nc.sync)
                load_half_cols("b", 1, 1, nc.sync)
            sb0, sb1 = 2 * sbp, 2 * sbp + 1
            # k and vT interleaved per-ec: both are wide [128c,512f] matmuls
            # sharing the same fresh x slice, so PE demand is a smooth
            # ~290GB/s. v is produced TRANSPOSED (like k) in 1/4 the PE
            # instructions of the per-128-col layout, then flipped to
            # [t, d] with 8 PE transposes per sbp. All psum accumulation
            # groups here are full-bank (512 cols) — concurrent groups in
            # one psum zero region are illegal and corrupt data.
            k_ps = ps.tile([128, 2 * SB], FP32, name=f"kps{r}_{sbp}",
                           tag="sps", bufs=2)
            v_ps = ps.tile([128, 2 * SB], FP32, name=f"vps{r}_{sbp}",
                           tag="sps", bufs=2)
            for ec in range(EC):
                nc.tensor.matmul(k_ps[:, 0:SB], wk_sb[:, ec, :],
                                 xt_slice("a", sb0, ec),
                                 start=(ec == 0), stop=(ec == EC - 1))
                nc.tensor.matmul(k_ps[:, SB:2 * SB], wk_sb[:, ec, :],
                                 xt_slice("a", sb1, ec),
                                 start=(ec == 0), stop=(ec == EC - 1))
                nc.tensor.matmul(v_ps[:, 0:SB], wv_sb[:, ec, :],
                                 xt_slice("a", sb0, ec),
                                 start=(ec == 0), stop=(ec == EC - 1))
                nc.tensor.matmul(v_ps[:, SB:2 * SB], wv_sb[:, ec, :],
                                 xt_slice("a", sb1, ec),
                                 start=(ec == 0), stop=(ec == EC - 1))
            nc.vector.tensor_copy(
                kT_sb[:, sb0 * SB:(sb0 + 2) * SB], k_ps[:])
            vT_t = vtp.tile([128, 2 * SB], FP16, name=f"vT{r}_{sbp}",
                            tag="vT")
            nc.scalar.copy(vT_t[:], v_ps[:])
            tr_ps = ps.tile([128, 2 * SB], FP16, name=f"trps{r}_{sbp}",
                            tag="kv")
            for j in range(2 * SB // 128):
                nc.tensor.transpose(tr_ps[:, j * 128:(j + 1) * 128],
                                    vT_t[:, j * 128:(j + 1) * 128],
                                    ident_sb[:])
            nc.vector.tensor_copy(
                v_sb[:, sb0 * SB:(sb0 + 2) * SB], tr_ps[:])



        def q_proj_closures(qb):
            """One closure per matmul of q(qb); evicts attached to the last."""
            cls = []
            for ic in range(HPC):
                q_ps = ps.tile([128, SB], FP32, name=f"qps{r}_{qb}_{ic}",
                               tag="q")

                def mk(ic, ec, q_ps):
                    def emit():
                        nc.tensor.matmul(
                            q_ps[:],
                            wq_sb[:, ec, ic * 128:(ic + 1) * 128],
                            xt_slice("b", qb, ec),
                            start=(ec == 0), stop=(ec == EC - 1))
                        if ec == EC - 1:
                            nc.vector.tensor_copy(
                                qT_sb[:, ic, qb * SB:(qb + 1) * SB], q_ps[:])
                    return emit
                for ec in range(EC):
                    cls.append(mk(ic, ec, q_ps))
            return cls

        n_y = [0]

        def o_proj_closures(qb, tag="kv"):
            """One closure per (sc, eb): 2 matmuls + evict; store per sc."""
            cls = []
            for sc in range(qb * (SB // 128), (qb + 1) * (SB // 128)):
                y_t = y_sbp.tile([128, E], FP16, name=f"ysb{r}_{sc}",
                                 tag="ysb")
                for eb in range(E // SB):
                    def mk(sc, eb, y_t):
                        def emit():
                            shape = [128, 2 * SB] if tag == "sps" else [128, SB]
                            y_ps = ps.tile(shape, FP32,
                                           name=f"yps{r}_{sc}_{eb}", tag=tag,
                                           bufs=2 if tag == "sps" else None)
                            for h in range(HPC):
                                nc.tensor.matmul(
                                    y_ps[:, 0:SB],
                                    aT_sb[:, h, sc * 128:(sc + 1) * 128],
                                    wo_sb[:, h, eb * SB:(eb + 1) * SB],
                                    start=(h == 0), stop=(h == HPC - 1))
                            nc.vector.tensor_copy(
                                y_t[:, eb * SB:(eb + 1) * SB], y_ps[:, 0:SB])
                            n_y[0] += 1
                            if sc == S // 128 - 1:
                                # last row block: store per-eb so the final
                                # DMA is small and the drain barrier clears
                                # sooner
                                nc.sync.dma_start(
                                    y[sc * 128:(sc + 1) * 128,
                                      eb * SB:(eb + 1) * SB],
                                    y_t[:, eb * SB:(eb + 1) * SB])
                            elif eb == E // SB - 1:
                                nc.sync.dma_start(
                                    y[sc * 128:(sc + 1) * 128, :], y_t[:])
                        return emit
                    cls.append(mk(sc, eb, y_t))
            return cls

        def finish_head_closure(h, qb, o_ps, sums):
            def emit():
                sums_ps = ps.tile([128, SB], FP32,
                                  name=f"sums_ps{r}_{h}_{qb}", tag="q")
                nc.tensor.matmul(sums_ps[:], ones_sb[:], sums[:],
                                 start=True, stop=True)
                recip = accp.tile([128, SB], FP32, name=f"recip{r}_{h}_{qb}",
                                  tag="recip")
                nc.vector.reciprocal(recip[:], sums_ps[:])
                nc.vector.tensor_mul(
                    aT_sb[:, h, qb * SB:(qb + 1) * SB], o_ps[:], recip[:])
            return emit

        # ---- Phase B: attention, with q/o-projection work as PE fillers ----
        from collections import deque
        fillers = deque()
        q0 = q_proj_closures(0)
        for c in q0[:EC]:        # q(0, head 0) must precede attention(h=0)
            c()
        fillers.extend(q0[EC:])  # q(0, head 1) fills attention(h=0) slack
        pending_oproj = []       # o_proj of qb becomes available after qb

        for qb in range(NSB):
            if qb + 1 < NSB:
                fillers.extend(q_proj_closures(qb + 1))
            fillers.extend(pending_oproj)
            pending_oproj = []
            for h in range(HPC):
                o_ps = ps.tile([128, SB], FP32, name=f"ops{r}_{h}_{qb}",
                               tag="osum", bufs=2)
                acc2 = accp.tile([128, 2 * SB], FP16, name=f"acc{r}_{h}_{qb}",
                                 tag="acc")

                def qk(tp):
                    s_ps = ps.tile([128, 2 * SB], FP32,
                                   name=f"sps{r}_{h}_{qb}_{tp}", tag="sps",
                                   bufs=2)
                    for hf in range(2):
                        t = tp * 2 + hf
                        nc.tensor.matmul(
                            s_ps[:, hf * SB:(hf + 1) * SB],
                            kT_sb[:, t * 128:(t + 1) * 128],
                            qT_sb[:, h, qb * SB:(qb + 1) * SB],
                            start=True, stop=True)
                    return s_ps

                s_prev = qk(0)
                pt_first = None
                for tp in range(TC // 2):
                    pt = ptp.tile([128, 2 * SB], FP16,
                                  name=f"pt{r}_{h}_{qb}_{tp}", tag="pt")
                    nc.scalar.activation(
                        pt[:], s_prev[:],
                        mybir.ActivationFunctionType.Exp, scale=SCALE)
                    if tp + 1 < TC // 2:
                        s_prev = qk(tp + 1)
                    for hf in range(2):
                        t = tp * 2 + hf
                        nc.tensor.matmul(
                            o_ps[:],
                            v_sb[:, t * 128:(t + 1) * 128],
                            pt[:, hf * SB:(hf + 1) * SB],
                            start=(t == 0), stop=(t == TC - 1))
                    if tp == 0:
                        pt_first = pt
                    elif tp == 1:
                        nc.vector.tensor_add(acc2[:], pt_first[:], pt[:])
                    else:
                        nc.vector.tensor_add(acc2[:], acc2[:], pt[:])
                    slots_left = (HPC - h) * (TC // 2) - tp
                    floor = 2 if (qb == 0 and h == 0) else 1
                    n_pop = min(len(fillers),
                                max(floor,
                                    -(-len(fillers) // max(slots_left, 1))),
                                3)
                    for _ in range(n_pop):
                        if fillers:
                            fillers.popleft()()
                sums = accp.tile([128, SB], FP16, name=f"sums{r}_{h}_{qb}",
                                 tag="sums")
                nc.vector.tensor_add(sums[:], acc2[:, 0:SB], acc2[:, SB:2 * SB])
                fillers.append(finish_head_closure(h, qb, o_ps, sums))
            pending_oproj = o_proj_closures(
                qb, tag="sps" if qb == NSB - 1 else "kv")

        while fillers:
            fillers.popleft()()
        for c in pending_oproj:
            c()

    big.release()


_WEIGHT_DMAS = []


def build_bass(reps=1):
    nc = bass.Bass("TRN2", target_bir_lowering=False, debug=False,
                   num_devices=N_CORES)
    xT = nc.dram_tensor("xT", [E, S], FP16, kind="ExternalInput").ap()
    wq = nc.dram_tensor("wq", [E, QDIM], FP16, kind="ExternalInput").ap()
    wk = nc.dram_tensor("wk", [E, D], FP16, kind="ExternalInput").ap()
    wv = nc.dram_tensor("wv", [E, D], FP16, kind="ExternalInput").ap()
    wo = nc.dram_tensor("wo", [QDIM, E], FP16, kind="ExternalInput").ap()
    y = nc.dram_tensor("y", [S, E], FP16, kind="ExternalOutput").ap()

    with tile.TileContext(nc) as tc, ExitStack() as ctx:
        wpool = ctx.enter_context(tc.tile_pool(name="wpool", bufs=1))
        wq_sb = wpool.tile([128, EC, QDIM], FP16)
        wk_sb = wpool.tile([128, EC, D], FP16)
        wv_sb = wpool.tile([128, EC, D], FP16)
        wo_sb = wpool.tile([128, HPC, E], FP16)
        ones_sb = wpool.tile([128, 128], FP16)
        nc.vector.memset(ones_sb[:], 1.0)
        ident_sb = wpool.tile([128, 128], FP16)
        masks.make_identity(nc, ident_sb[:])
        kv_dmas = [
            (wk_sb[:], wk.rearrange("(ec p) d -> p ec d", p=128)),
            (wv_sb[:], wv.rearrange("(ec p) d -> p ec d", p=128)),
        ]
        qo_dmas = [
            (wq_sb[:], wq.rearrange("(ec p) d -> p ec d", p=128)),
            (wo_sb[:], wo.rearrange("(h p) e -> p h e", p=128)),
        ]
        _WEIGHT_DMAS.clear()
        _WEIGHT_DMAS.append({"kv": kv_dmas, "qo": qo_dmas})

        for r in range(reps):
            _emit_program(nc, tc, (xT, y),
                          (wq_sb, wk_sb, wv_sb, wo_sb, ones_sb, ident_sb), r)

    _split_sync_waits(nc)
    return nc


def make_in_maps(x, Wq, Wk, Wv, Wo):
    """Host-side sharding: transpose/cast to fp16, slice weights per core."""
    x = np.asarray(x, dtype=np.float32).reshape(S, E)
    xT = np.ascontiguousarray(x.T).astype(np.float16)
    WqT = np.ascontiguousarray(np.asarray(Wq, dtype=np.float32).T).astype(np.float16)
    WkT = np.ascontiguousarray(np.asarray(Wk, dtype=np.float32).T).astype(np.float16)
    WvT = np.ascontiguousarray(np.asarray(Wv, dtype=np.float32).T).astype(np.float16)
    WoT = np.ascontiguousarray(np.asarray(Wo, dtype=np.float32).T).astype(np.float16)
    in_maps = []
    for c in range(N_CORES):
        g = (c * HPC) // (H // HK)      # kv head for this core's q heads
        in_maps.append({
            "xT": xT,
            "wq": np.ascontiguousarray(WqT[:, c * QDIM:(c + 1) * QDIM]),
            "wk": np.ascontiguousarray(WkT[:, g * D:(g + 1) * D]),
            "wv": np.ascontiguousarray(WvT[:, g * D:(g + 1) * D]),
            "wo": np.ascontiguousarray(WoT[c * QDIM:(c + 1) * QDIM, :]),
        })
    return in_maps


class Runner:
    """Compile the bass program once and keep the jitted SPMD executable
    alive, so repeated kernel()/timing calls skip re-trace + re-compile
    (the stock run_bass_kernel_spmd path builds a fresh closure per call,
    costing ~10 s each). Mirrors bass2jax.run_bass_via_pjrt's staging."""

    def __init__(self, reps=1):
        import jax
        from jax.sharding import PartitionSpec, Mesh, NamedSharding
        from jax.experimental.shard_map import shard_map
        from concourse import bass2jax

        self.jax = jax
        nc = build_bass(reps)
        self.nc = nc
        bass2jax.install_neuronx_cc_hook()
        partition_name = (nc.partition_id_tensor.name
                          if nc.partition_id_tensor else None)
        in_names, out_names, out_avals, zero_outs = [], [], [], []
        for alloc in nc.m.functions[0].allocations:
            if not isinstance(alloc, mybir.MemoryLocationSet):
                continue
            name = alloc.memorylocations[0].name
            if alloc.kind == "ExternalInput":
                if name != partition_name:
                    in_names.append(name)
            elif alloc.kind == "ExternalOutput":
                out_names.append(name)
                shape = tuple(alloc.tensor_shape)
                dtype = mybir.dt.np(alloc.dtype)
                out_avals.append(jax.core.ShapedArray(shape, dtype))
                zero_outs.append(np.zeros(shape, dtype))
        self.in_names, self.out_names = in_names, out_names
        self.zero_outs = zero_outs
        all_names = in_names + out_names
        if partition_name is not None:
            all_names = all_names + [partition_name]

        def _body(*args):
            operands = list(args)
            if partition_name is not None:
                operands.append(bass2jax.partition_id_tensor())
            outs = bass2jax._bass_exec_p.bind(
                *operands,
                out_avals=tuple(out_avals),
                in_names=tuple(all_names),
                out_names=tuple(out_names),
                lowering_input_output_aliases=(),
                sim_require_finite=True,
                sim_require_nnan=True,
                nc=nc,
            )
            return tuple(outs)

        devices = jax.devices()[:N_CORES]
        mesh = Mesh(np.asarray(devices), ("core",))
        spec = PartitionSpec("core")
        self.sharding = NamedSharding(mesh, spec)
        self.fn = jax.jit(shard_map(_body, mesh=mesh, in_specs=spec,
                                    out_specs=spec, check_rep=False))

    def stage(self, in_maps):
        """Concat per-core inputs on axis 0 and put on the 8 devices."""
        global_ins = [
            np.concatenate([np.asarray(m[name]) for m in in_maps], axis=0)
            for name in self.in_names
        ] + [
            np.concatenate([z] * N_CORES, axis=0) for z in self.zero_outs
        ]
        return [self.jax.device_put(a, self.sharding) for a in global_ins]

    def run_staged(self, dev_ins):
        return self.fn(*dev_ins)

    def run(self, in_maps):
        outs = self.fn(*self.stage(in_maps))
        return [
            {name: np.asarray(outs[i]).reshape(N_CORES, *self.zero_outs[i].shape)[c]
             for i, name in enumerate(self.out_names)}
            for c in range(N_CORES)
        ]


_RUNNER_CACHE = {}


def get_runner(reps=1):
    if reps not in _RUNNER_CACHE:
        _RUNNER_CACHE[reps] = Runner(reps)
    return _RUNNER_CACHE[reps]


def get_nc():
    return get_runner(1).nc


def kernel(x, Wq, Wk, Wv, Wo):
    runner = get_runner(1)
    in_maps = make_in_maps(x, Wq, Wk, Wv, Wo)
    res = runner.run(in_maps)
    out = np.zeros((S, E), dtype=np.float32)
    for r in res:
        out += r["y"].astype(np.float32)
    return out.reshape(B, S, E)



# revision 24
# speedup vs baseline: 1.1168x; 1.1117x over previous
"""GQA multi-head attention (B=1, S=4096, E=2048, H=16, HK=4, D=128) on 8 trn2
NeuronCores.

Sharding: tensor-parallel over query heads — 2 q-heads per core, each core
also computes the kv head its q-heads attend to (each kv head is replicated
on the 2 cores that need it). Each core produces a partial output
y_c = attn_c @ Wo_c and the host sums the 8 partials during unsharding
(so the device program needs no collectives).

Device-side dataflow per core (matmul inputs fp16, accumulation fp32):
  xT [E,S] -> qT [D,h,S], kT [D,S], vT [D,S] (all transposed projections,
    wide [128c,512f] matmuls; v then flipped to v [t%128, tc*128+d] with
    PE identity-transposes, 8 per 1024-column block)
  scoresT[t,sq] = (kT chunk as lhsT).T @ qT      (t-chunk on partitions)
  pT = exp(scoresT/sqrt(D)) via ACT -> fp16
  outT[d,sq] accumulated over t-chunks: lhsT=v[t,d], rhs=pT[t,sq]
  rowsums: DVE adds over t-chunks, then ones-matmul partition-sum+broadcast
  attnT = outT * (1/rowsum); o_proj: y[s,e] = (attnT as lhsT).T @ WoT

Schedule notes (see _emit_program): x is streamed in consumption order as
[128,1024] column-slices dual-issued from SP and ACT (DMA issue costs
~0.8us/slice on the issuing engine and would otherwise pace phase A); k and
vT matmuls interleave per e-chunk so fresh-x demand stays ~290GB/s; psum
accumulation groups are kept full-bank (concurrent groups within one
512-col psum zero region are illegal and corrupt results on HW).
"""
import math
import numpy as np
from contextlib import ExitStack

import concourse.bass as bass
import concourse.mybir as mybir
from concourse import tile
from concourse import bass_utils
from concourse import masks

B, S, E = 1, 4096, 2048
H, HK, D = 16, 4, 128
N_CORES = 8
HPC = H // N_CORES          # q heads per core
QDIM = HPC * D              # 256
EC = E // 128               # e-chunks
SB = 512                    # s/sq block
NSB = S // SB
TC = S // 128               # t-chunks
SCALE = 1.0 / math.sqrt(D)
FP16 = mybir.dt.float16
FP32 = mybir.dt.float32


def _split_sync_waits(nc, cap=1):
    """This container's walrus build rejects instructions carrying more than
    ~1 sync-wait (codegen 'Too many sync wait commands'). Post-pass over the
    scheduled BIR: for any instruction with >cap waits, hoist the excess onto
    same-engine NOPs inserted immediately before it (same block, so per-engine
    program order — and therefore semantics — is preserved)."""
    n = 0
    for fn in nc.m.functions:
        for blk in fn.blocks:
            il = blk.instructions
            i = 0
            while i < len(il):
                inst = il[i]
                si = getattr(inst, "sync_info", None)
                if si is not None and len(si.on_wait) > cap:
                    waits = list(si.on_wait)
                    si.on_wait = waits[-cap:]
                    extras = []
                    for w in waits[:-cap]:
                        nop = mybir.InstNoOp(name=f"I-waitfix-{n}", ins=[], outs=[])
                        n += 1
                        nop.engine = inst.engine
                        nop.sync_info = mybir.SyncInfo(on_wait=[w], on_update=[])
                        extras.append(nop)
                    il[i:i] = extras
                    i += len(extras)
                i += 1
    return n


XTW = 2048                  # xt tile width (half of S per tile)
NHALF = S // XTW            # 2 halves


def _emit_program(nc, tc, aps, weights, r):
    """Emit one full forward pass. `r` suffixes pool/tile names so the
    program can be repeated for timing calibration.

    The PE executes its instruction stream in order, so emission order is
    the PE schedule. The attention inner loop is software-pipelined
    (QK of chunk tp+1 is emitted before PV of chunk tp, so the PE never
    waits on the exp of the chunk it just produced), and independent
    matmuls (q-projection of the next query block, output projection of
    the previous one, rowsum matmuls) are injected as fillers into the
    slack the ACT-paced exp stream leaves on the PE.

    PSUM budget (8 banks): kv/y shared tag 1, q 1, sps 2x2, osum 2x1."""
    xT, y = aps
    wq_sb, wk_sb, wv_sb, wo_sb, ones_sb, ident_sb = weights

    big = tc.alloc_tile_pool(name=f"big{r}", bufs=1)
    qT_sb = big.tile([128, HPC, S], FP16, name=f"qT{r}")   # [d, h, s]
    kT_sb = big.tile([128, S], FP16, name=f"kT{r}")        # [d, t]
    v_sb = big.tile([128, S], FP16, name=f"v{r}")          # [t%128, tc*128+d]
    aT_sb = big.tile([128, HPC, S], FP16, name=f"aT{r}")   # [d, h, s]

    with ExitStack() as ctx:
        xpool = ctx.enter_context(tc.tile_pool(name=f"xpool{r}", bufs=24))
        ps = ctx.enter_context(tc.tile_pool(name=f"ps{r}", bufs=1, space="PSUM"))
        ptp = ctx.enter_context(tc.tile_pool(name=f"ptp{r}", bufs=4))
        accp = ctx.enter_context(tc.tile_pool(name=f"accp{r}", bufs=2))
        y_sbp = ctx.enter_context(tc.tile_pool(name=f"y_sbp{r}", bufs=2))
        vtp = ctx.enter_context(tc.tile_pool(name=f"vtp{r}", bufs=2))

        xt_tiles = {}
        HW = XTW // 2           # column half of one xt tile (2 sb blocks)

        def load_half_cols(phase, half, part, eng):
            """DMA the `part` column-half of all EC tiles of (phase, half),
            in ec order — i.e. in PE consumption order, so the k/v/q matmul
            ec-sweep never waits on data that arrives later than needed.
            `eng` picks the issuing engine: DMA issue costs ~0.8us per slice
            on the issuing engine, so x streaming is spread across SP and
            ACT (idle during phase A) to keep issue off the critical path."""
            for ec in range(EC):
                key = (phase, half, ec)
                if key not in xt_tiles:
                    xt_tiles[key] = xpool.tile(
                        [128, XTW], FP16,
                        name=f"xt{r}{phase}_{half}_{ec}", tag="xt")
                src0 = half * XTW + part * HW
                eng.dma_start(
                    xt_tiles[key][:, part * HW:(part + 1) * HW],
                    xT[ec * 128:(ec + 1) * 128, src0:src0 + HW])

        def load_full(phase, half, eng):
            """Whole-tile loads (4KB per-partition lines — best DMA
            efficiency) for tiles that are consumed slowly (q-projection
            stream), where arrival pacing doesn't matter."""
            for ec in range(EC):
                key = (phase, half, ec)
                assert key not in xt_tiles
                xt_tiles[key] = xpool.tile(
                    [128, XTW], FP16,
                    name=f"xt{r}{phase}_{half}_{ec}", tag="xt")
                eng.dma_start(
                    xt_tiles[key][:],
                    xT[ec * 128:(ec + 1) * 128,
                       half * XTW:(half + 1) * XTW])

        def xt_slice(phase, sb, ec, width=SB, sub=0):
            half, off = divmod(sb * SB + sub, XTW)
            return xt_tiles[(phase, half, ec)][:, off:off + width]

        # ---- Phase A: k/v projections ----
        # Consumption-ordered DMAs: first wk chunk first (the first matmul
        # needs only wk[:, 0:4, :]), x column-halves in sbp consumption
        # order, dual-issued on SP + ACT.
        if r == 0:
            wdmas = _WEIGHT_DMAS.pop(0)
            (wk_dst, wk_src), (wv_dst, wv_src) = wdmas["kv"]
            (wq_dst, wq_src), (wo_dst, wo_src) = wdmas["qo"]
            for g in range(0, EC, 4):       # wk in 4-ec chunks
                nc.sync.dma_start(wk_dst[:, g:g + 4, :], wk_src[:, g:g + 4, :])
            for g in range(0, EC, 8):       # wv in 8-ec chunks
                nc.sync.dma_start(wv_dst[:, g:g + 8, :], wv_src[:, g:g + 8, :])
        load_half_cols("a", 0, 0, nc.scalar)    # sbp 0
        load_half_cols("a", 0, 1, nc.sync)      # sbp 1

        for sbp in range(NSB // 2):
            if sbp == 1:
                load_half_cols("a", 1, 0, nc.scalar)
                load_half_cols("a", 1, 1, nc.sync)
            if sbp == 2:
                if r == 0:
                    for g in range(0, EC, 8):
                        nc.scalar.dma_start(wq_dst[:, g:g + 8, :],
                                            wq_src[:, g:g + 8, :])
                load_full("b", 0, nc.sync)
            if sbp == 3:
                if r == 0:
                    nc.sync.dma_start(wo_dst, wo_src)
                load_full("b", 1, nc.sync)
            sb0, sb1 = 2 * sbp, 2 * sbp + 1
            # k and vT interleaved per-ec: both are wide [128c,512f] matmuls
            # sharing the same fresh x slice, so PE demand is a smooth
            # ~290GB/s. v is produced TRANSPOSED (like k) in 1/4 the PE
            # instructions of the per-128-col layout, then flipped to
            # [t, d] with 8 PE transposes per sbp. All psum accumulation
            # groups here are full-bank (512 cols) — concurrent groups in
            # one psum zero region are illegal and corrupt data.
            k_ps = ps.tile([128, 2 * SB], FP32, name=f"kps{r}_{sbp}",
                           tag="sps", bufs=2)
            v_ps = ps.tile([128, 2 * SB], FP32, name=f"vps{r}_{sbp}",
                           tag="sps", bufs=2)
            for ec in range(EC):
                nc.tensor.matmul(k_ps[:, 0:SB], wk_sb[:, ec, :],
                                 xt_slice("a", sb0, ec),
                                 start=(ec == 0), stop=(ec == EC - 1))
                nc.tensor.matmul(k_ps[:, SB:2 * SB], wk_sb[:, ec, :],
                                 xt_slice("a", sb1, ec),
                                 start=(ec == 0), stop=(ec == EC - 1))
                nc.tensor.matmul(v_ps[:, 0:SB], wv_sb[:, ec, :],
                                 xt_slice("a", sb0, ec),
                                 start=(ec == 0), stop=(ec == EC - 1))
                nc.tensor.matmul(v_ps[:, SB:2 * SB], wv_sb[:, ec, :],
                                 xt_slice("a", sb1, ec),
                                 start=(ec == 0), stop=(ec == EC - 1))
            nc.vector.tensor_copy(
                kT_sb[:, sb0 * SB:(sb0 + 2) * SB], k_ps[:])
            vT_t = vtp.tile([128, 2 * SB], FP16, name=f"vT{r}_{sbp}",
                            tag="vT")
            nc.scalar.copy(vT_t[:], v_ps[:])
            tr_ps = ps.tile([128, 2 * SB], FP16, name=f"trps{r}_{sbp}",
                            tag="kv")
            for j in range(2 * SB // 128):
                nc.tensor.transpose(tr_ps[:, j * 128:(j + 1) * 128],
                                    vT_t[:, j * 128:(j + 1) * 128],
                                    ident_sb[:])
            nc.vector.tensor_copy(
                v_sb[:, sb0 * SB:(sb0 + 2) * SB], tr_ps[:])



        def q_proj_closures(qb):
            """One closure per matmul of q(qb); evicts attached to the last."""
            cls = []
            for ic in range(HPC):
                q_ps = ps.tile([128, SB], FP32, name=f"qps{r}_{qb}_{ic}",
                               tag="q")

                def mk(ic, ec, q_ps):
                    def emit():
                        nc.tensor.matmul(
                            q_ps[:],
                            wq_sb[:, ec, ic * 128:(ic + 1) * 128],
                            xt_slice("b", qb, ec),
                            start=(ec == 0), stop=(ec == EC - 1))
                        if ec == EC - 1:
                            nc.vector.tensor_copy(
                                qT_sb[:, ic, qb * SB:(qb + 1) * SB], q_ps[:])
                    return emit
                for ec in range(EC):
                    cls.append(mk(ic, ec, q_ps))
            return cls

        n_y = [0]

        def o_proj_closures(qb, tag="kv"):
            """One closure per (sc, eb): 2 matmuls + evict; store per sc."""
            cls = []
            for sc in range(qb * (SB // 128), (qb + 1) * (SB // 128)):
                y_t = y_sbp.tile([128, E], FP16, name=f"ysb{r}_{sc}",
                                 tag="ysb")
                for eb in range(E // SB):
                    def mk(sc, eb, y_t):
                        def emit():
                            shape = [128, 2 * SB] if tag == "sps" else [128, SB]
                            y_ps = ps.tile(shape, FP32,
                                           name=f"yps{r}_{sc}_{eb}", tag=tag,
                                           bufs=2 if tag == "sps" else None)
                            for h in range(HPC):
                                nc.tensor.matmul(
                                    y_ps[:, 0:SB],
                                    aT_sb[:, h, sc * 128:(sc + 1) * 128],
                                    wo_sb[:, h, eb * SB:(eb + 1) * SB],
                                    start=(h == 0), stop=(h == HPC - 1))
                            nc.vector.tensor_copy(
                                y_t[:, eb * SB:(eb + 1) * SB], y_ps[:, 0:SB])
                            n_y[0] += 1
                            if sc == S // 128 - 1:
                                # last row block: store per-eb so the final
                                # DMA is small and the drain barrier clears
                                # sooner
                                nc.sync.dma_start(
                                    y[sc * 128:(sc + 1) * 128,
                                      eb * SB:(eb + 1) * SB],
                                    y_t[:, eb * SB:(eb + 1) * SB])
                            elif eb == E // SB - 1:
                                nc.sync.dma_start(
                                    y[sc * 128:(sc + 1) * 128, :], y_t[:])
                        return emit
                    cls.append(mk(sc, eb, y_t))
            return cls

        def finish_head_closure(h, qb, o_ps, sums):
            def emit():
                sums_ps = ps.tile([128, SB], FP32,
                                  name=f"sums_ps{r}_{h}_{qb}", tag="q")
                nc.tensor.matmul(sums_ps[:], ones_sb[:], sums[:],
                                 start=True, stop=True)
                recip = accp.tile([128, SB], FP32, name=f"recip{r}_{h}_{qb}",
                                  tag="recip")
                nc.vector.reciprocal(recip[:], sums_ps[:])
                nc.vector.tensor_mul(
                    aT_sb[:, h, qb * SB:(qb + 1) * SB], o_ps[:], recip[:])
            return emit

        # ---- Phase B: attention, with q/o-projection work as PE fillers ----
        from collections import deque
        fillers = deque()
        q0 = q_proj_closures(0)
        for c in q0[:EC]:        # q(0, head 0) must precede attention(h=0)
            c()
        fillers.extend(q0[EC:])  # q(0, head 1) fills attention(h=0) slack
        pending_oproj = []       # o_proj of qb becomes available after qb

        for qb in range(NSB):
            if qb + 1 < NSB:
                fillers.extend(q_proj_closures(qb + 1))
            fillers.extend(pending_oproj)
            pending_oproj = []
            for h in range(HPC):
                o_ps = ps.tile([128, SB], FP32, name=f"ops{r}_{h}_{qb}",
                               tag="osum", bufs=2)
                acc2 = accp.tile([128, 2 * SB], FP16, name=f"acc{r}_{h}_{qb}",
                                 tag="acc")

                def qk(tp):
                    s_ps = ps.tile([128, 2 * SB], FP32,
                                   name=f"sps{r}_{h}_{qb}_{tp}", tag="sps",
                                   bufs=2)
                    for hf in range(2):
                        t = tp * 2 + hf
                        nc.tensor.matmul(
                            s_ps[:, hf * SB:(hf + 1) * SB],
                            kT_sb[:, t * 128:(t + 1) * 128],
                            qT_sb[:, h, qb * SB:(qb + 1) * SB],
                            start=True, stop=True)
                    return s_ps

                s_prev = qk(0)
                pt_first = None
                for tp in range(TC // 2):
                    pt = ptp.tile([128, 2 * SB], FP16,
                                  name=f"pt{r}_{h}_{qb}_{tp}", tag="pt")
                    nc.scalar.activation(
                        pt[:], s_prev[:],
                        mybir.ActivationFunctionType.Exp, scale=SCALE)
                    if tp + 1 < TC // 2:
                        s_prev = qk(tp + 1)
                    for hf in range(2):
                        t = tp * 2 + hf
                        nc.tensor.matmul(
                            o_ps[:],
                            v_sb[:, t * 128:(t + 1) * 128],
                            pt[:, hf * SB:(hf + 1) * SB],
                            start=(t == 0), stop=(t == TC - 1))
                    if tp == 0:
                        pt_first = pt
                    elif tp == 1:
                        nc.vector.tensor_add(acc2[:], pt_first[:], pt[:])
                    else:
                        nc.vector.tensor_add(acc2[:], acc2[:], pt[:])
                    slots_left = (HPC - h) * (TC // 2) - tp
                    floor = 2 if (qb == 0 and h == 0) else 1
                    n_pop = min(len(fillers),
                                max(floor,
                                    -(-len(fillers) // max(slots_left, 1))),
                                3)
                    for _ in range(n_pop):
                        if fillers:
                            fillers.popleft()()
                sums = accp.tile([128, SB], FP16, name=f"sums{r}_{h}_{qb}",
                                 tag="sums")
                nc.vector.tensor_add(sums[:], acc2[:, 0:SB], acc2[:, SB:2 * SB])
                fillers.append(finish_head_closure(h, qb, o_ps, sums))
            pending_oproj = o_proj_closures(
                qb, tag="sps" if qb == NSB - 1 else "kv")

        while fillers:
            fillers.popleft()()
        for c in pending_oproj:
            c()

    big.release()


_WEIGHT_DMAS = []


def build_bass(reps=1):
    nc = bass.Bass("TRN2", target_bir_lowering=False, debug=False,
                   num_devices=N_CORES)
    xT = nc.dram_tensor("xT", [E, S], FP16, kind="ExternalInput").ap()
    wq = nc.dram_tensor("wq", [E, QDIM], FP16, kind="ExternalInput").ap()
    wk = nc.dram_tensor("wk", [E, D], FP16, kind="ExternalInput").ap()
    wv = nc.dram_tensor("wv", [E, D], FP16, kind="ExternalInput").ap()
    wo = nc.dram_tensor("wo", [QDIM, E], FP16, kind="ExternalInput").ap()
    y = nc.dram_tensor("y", [S, E], FP16, kind="ExternalOutput").ap()

    with tile.TileContext(nc) as tc, ExitStack() as ctx:
        wpool = ctx.enter_context(tc.tile_pool(name="wpool", bufs=1))
        wq_sb = wpool.tile([128, EC, QDIM], FP16)
        wk_sb = wpool.tile([128, EC, D], FP16)
        wv_sb = wpool.tile([128, EC, D], FP16)
        wo_sb = wpool.tile([128, HPC, E], FP16)
        ones_sb = wpool.tile([128, 128], FP16)
        nc.vector.memset(ones_sb[:], 1.0)
        ident_sb = wpool.tile([128, 128], FP16)
        masks.make_identity(nc, ident_sb[:])
        kv_dmas = [
            (wk_sb[:], wk.rearrange("(ec p) d -> p ec d", p=128)),
            (wv_sb[:], wv.rearrange("(ec p) d -> p ec d", p=128)),
        ]
        qo_dmas = [
            (wq_sb[:], wq.rearrange("(ec p) d -> p ec d", p=128)),
            (wo_sb[:], wo.rearrange("(h p) e -> p h e", p=128)),
        ]
        _WEIGHT_DMAS.clear()
        _WEIGHT_DMAS.append({"kv": kv_dmas, "qo": qo_dmas})

        for r in range(reps):
            _emit_program(nc, tc, (xT, y),
                          (wq_sb, wk_sb, wv_sb, wo_sb, ones_sb, ident_sb), r)

    _split_sync_waits(nc)
    return nc


def make_in_maps(x, Wq, Wk, Wv, Wo):
    """Host-side sharding: transpose/cast to fp16, slice weights per core."""
    x = np.asarray(x, dtype=np.float32).reshape(S, E)
    xT = np.ascontiguousarray(x.T).astype(np.float16)
    WqT = np.ascontiguousarray(np.asarray(Wq, dtype=np.float32).T).astype(np.float16)
    WkT = np.ascontiguousarray(np.asarray(Wk, dtype=np.float32).T).astype(np.float16)
    WvT = np.ascontiguousarray(np.asarray(Wv, dtype=np.float32).T).astype(np.float16)
    WoT = np.ascontiguousarray(np.asarray(Wo, dtype=np.float32).T).astype(np.float16)
    in_maps = []
    for c in range(N_CORES):
        g = (c * HPC) // (H // HK)      # kv head for this core's q heads
        in_maps.append({
            "xT": xT,
            "wq": np.ascontiguousarray(WqT[:, c * QDIM:(c + 1) * QDIM]),
            "wk": np.ascontiguousarray(WkT[:, g * D:(g + 1) * D]),
            "wv": np.ascontiguousarray(WvT[:, g * D:(g + 1) * D]),
            "wo": np.ascontiguousarray(WoT[c * QDIM:(c + 1) * QDIM, :]),
        })
    return in_maps


class Runner:
    """Compile the bass program once and keep the jitted SPMD executable
    alive, so repeated kernel()/timing calls skip re-trace + re-compile
    (the stock run_bass_kernel_spmd path builds a fresh closure per call,
    costing ~10 s each). Mirrors bass2jax.run_bass_via_pjrt's staging."""

    def __init__(self, reps=1):
        import jax
        from jax.sharding import PartitionSpec, Mesh, NamedSharding
        from jax.experimental.shard_map import shard_map
        from concourse import bass2jax

        self.jax = jax
        nc = build_bass(reps)
        self.nc = nc
        bass2jax.install_neuronx_cc_hook()
        partition_name = (nc.partition_id_tensor.name
                          if nc.partition_id_tensor else None)
        in_names, out_names, out_avals, zero_outs = [], [], [], []
        for alloc in nc.m.functions[0].allocations:
            if not isinstance(alloc, mybir.MemoryLocationSet):
                continue
            name = alloc.memorylocations[0].name
            if alloc.kind == "ExternalInput":
                if name != partition_name:
                    in_names.append(name)
            elif alloc.kind == "ExternalOutput":
                out_names.append(name)
                shape = tuple(alloc.tensor_shape)
                dtype = mybir.dt.np(alloc.dtype)
                out_avals.append(jax.core.ShapedArray(shape, dtype))
                zero_outs.append(np.zeros(shape, dtype))
        self.in_names, self.out_names = in_names, out_names
        self.zero_outs = zero_outs
        all_names = in_names + out_names
        if partition_name is not None:
            all_names = all_names + [partition_name]

        def _body(*args):
            operands = list(args)
            if partition_name is not None:
                operands.append(bass2jax.partition_id_tensor())
            outs = bass2jax._bass_exec_p.bind(
                *operands,
                out_avals=tuple(out_avals),
                in_names=tuple(all_names),
                out_names=tuple(out_names),
                lowering_input_output_aliases=(),
                sim_require_finite=True,
                sim_require_nnan=True,
                nc=nc,
            )
            return tuple(outs)

        devices = jax.devices()[:N_CORES]
        mesh = Mesh(np.asarray(devices), ("core",))
        spec = PartitionSpec("core")
        self.sharding = NamedSharding(mesh, spec)
        self.fn = jax.jit(shard_map(_body, mesh=mesh, in_specs=spec,
                                    out_specs=spec, check_rep=False))

    def stage(self, in_maps):
        """Concat per-core inputs on axis 0 and put on the 8 devices."""
        global_ins = [
            np.concatenate([np.asarray(m[name]) for m in in_maps], axis=0)
            for name in self.in_names
        ] + [
            np.concatenate([z] * N_CORES, axis=0) for z in self.zero_outs
        ]
        return [self.jax.device_put(a, self.sharding) for a in global_ins]

    def run_staged(self, dev_ins):
        return self.fn(*dev_ins)

    def run(self, in_maps):
        outs = self.fn(*self.stage(in_maps))
        return [
            {name: np.asarray(outs[i]).reshape(N_CORES, *self.zero_outs[i].shape)[c]
             for i, name in enumerate(self.out_names)}
            for c in range(N_CORES)
        ]


_RUNNER_CACHE = {}


def get_runner(reps=1):
    if reps not in _RUNNER_CACHE:
        _RUNNER_CACHE[reps] = Runner(reps)
    return _RUNNER_CACHE[reps]


def get_nc():
    return get_runner(1).nc


def kernel(x, Wq, Wk, Wv, Wo):
    runner = get_runner(1)
    in_maps = make_in_maps(x, Wq, Wk, Wv, Wo)
    res = runner.run(in_maps)
    out = np.zeros((S, E), dtype=np.float32)
    for r in res:
        out += r["y"].astype(np.float32)
    return out.reshape(B, S, E)

